# revision 82
# baseline (speedup 1.0000x reference)
"""Distributed Trainium2 Bass kernel for nn_Attention_87368224735328.

reference:
    score = einsum("bqd,bkd->bqk", enc_outputs, atten_outputs)   # [B,S1,S2]
    alignment = softmax(score, axis=-1)                          # over S2
    out = einsum("bqk,bqd->bkd", alignment, enc_outputs + enc_residual)

Sharding: 8 cores = (batch b in 0..3) x (S2-half in 0..1). Each core computes
its local [S1, S2/2] score block, local softmax row-stats (max / sum-exp over
its S2 half), exchanges the tiny [S1] stats with its partner core, and runs
the second GEMM fully locally (contraction over S1 is complete on every
core). Output shard: [S2/2, D] -> out[b, half].

The kernel is TensorEngine-bound (fp16, 2 x 131072 PE cycles ~ 109 us at
2.4 GHz). Schedule principles (vs the earlier revision):

- DMA consolidation: enc/res ride 4+4 big grouped DMAs (paced behind early
  exps via WAW markers written into their destinations, since plain DMAs
  are issue-hoisted to the queue front), qt's second wave is 8 chunk DMAs
  strictly after wave 1, so the shared HWDGE descriptor-gen device (~630 ns
  per DMA) and the single serialized DMA-transfer device never starve the
  GEMM1 operand stream. This also pulls exchange A's send (and hence its
  collective + merge + all A-half v-scales) ~10 us earlier, completely off
  the GEMM1->GEMM2 seam.
- V = enc + res adds run on the otherwise-idle Pool engine, keeping DVE
  clear for the reduce->exp->exchange chains.
- reduce15/exp15 run at their natural times (no v-scale block queued ahead
  of them on DVE/Act), so exchange Z's collective starts early and the
  phase-1 gate has slack; the ramp staircase runs chunk-ascending within
  its first steps so a late-arriving chunk never blocks ready matmuls.
- GEMM2 is split in phase 0 (q tiles [0,8), needs only exchange A) and
  phase 1 ([8,16), needs Z), with phase-0 partials spilled to SBUF and
  added back on the way out (PSUM only holds 4 x [128,1024] f32).
- Output is fp16 on device (halves the out DMA bytes; host casts to f32;
  quantization ~2.4e-4 rel, far below the 1.6e-3 operand-precision floor),
  with out DMAs alternating between the sync and scalar queues; the last
  k tile drains through four 256-column PSUM quarters so only a small
  add + store chain trails the final matmul.

Precision: fp16 operands on the TensorEngine, f32 PSUM accumulation,
stats/softmax math in f32. Measured end-to-end rel err ~1.6e-3.
"""

import numpy as np

from concourse import bacc, mybir, tile
from concourse.bass_utils import run_bass_kernel_spmd

B, S, D = 4, 2048, 1024
S2L = S // 2          # local S2 columns per core
NQT = S // 128        # 16 q tiles (S1)
NDC = D // 128        # 8 contraction chunks for GEMM1
NKB = S2L // 512      # 2 PSUM blocks of 512 for GEMM1
NKT = S2L // 128      # 8 output k tiles for GEMM2
SPLIT = 8             # q-tile boundary between exchange A and exchange Z
NWARM = 11            # dummy warm-up matmuls holding the PE p-state
FP16 = mybir.dt.float16
F32 = mybir.dt.float32
N_CORES = 8
RGP = [[0, 1], [2, 3], [4, 5], [6, 7]]


def _emit_stats_send(nc, P, DR, sel2_sb, negm, zloc, lo, hi, tag,
                     use_collective):
    """Ship local (-m, z) for q tiles [lo, hi) to the partner core via a
    send-side-masked pairwise ReduceScatter: stats_in has TWO 128-row slots
    (ReduceScatter distributes along partitions in p_dim/group slabs); each
    core writes its stats into the slot its PARTNER will receive (one-hot
    sel2 mask on parity) and zeros into its own, so the summed slot a core
    receives IS the partner's stats -- no rank-indexed gather or select on
    the post-collective path. Returns the [128, 2n] DRAM output."""
    n = hi - lo
    msk = P.tile([128, 4 * n], F32, tag=f"msk{tag}", name=f"msk{tag}")
    for r in range(2):
        nc.vector.tensor_scalar_mul(
            out=msk[:, r * 2 * n:r * 2 * n + n], in0=negm[:, lo:hi],
            scalar1=sel2_sb[:, r:r + 1])
        nc.vector.tensor_scalar_mul(
            out=msk[:, r * 2 * n + n:(r + 1) * 2 * n], in0=zloc[:, lo:hi],
            scalar1=sel2_sb[:, r:r + 1])
    stats_in = DR.tile([2 * 128, 2 * n], F32, tag=f"si{tag}",
                       name=f"stats_in{tag}")
    stats_out = DR.tile([128, 2 * n], F32, tag=f"so{tag}",
                        name=f"stats_out{tag}")
    for r in range(2):
        nc.sync.dma_start(out=stats_in[r * 128:(r + 1) * 128, :],
                          in_=msk[:, r * 2 * n:(r + 1) * 2 * n])
    if use_collective:
        nc.gpsimd.collective_compute(
            "ReduceScatter", mybir.AluOpType.add,
            replica_groups=RGP,
            ins=[stats_in[:, :].opt()],
            outs=[stats_out[:, :].opt()],
            cc_dim="Partition",
        )
    else:  # debug/sim variant: self-merge via slot sum
        nc.gpsimd.dma_start(out=stats_out[:, :], in_=msk[:, 0:2 * n])
    return stats_out


def _emit_stats_recv_dma(nc, P, stats_out, n, tag):
    acc = P.tile([128, 2 * n], F32, tag=f"acc{tag}", name=f"acc{tag}")
    nc.sync.dma_start(out=acc[:, :], in_=stats_out[:, :])
    return acc


def _emit_stats_merge(nc, P, acc, negm, zloc, cs, lo, hi, tag):
    """cs = 1/(z0 + exp(n0 - n1) * z1) with n_i = -m_i: one exp, no global
    max needed. exp overflow (partner max >> local max) saturates to inf ->
    cs = 0, the correct limit; underflow -> 1/z0.
    DVE: sub, mul, add, recip; Act: exp."""
    n = hi - lo
    d = P.tile([128, n], F32, tag=f"d{tag}", name=f"d{tag}")
    t = P.tile([128, n], F32, tag=f"t{tag}", name=f"t{tag}")
    zg = P.tile([128, n], F32, tag=f"zg{tag}", name=f"zg{tag}")
    nc.vector.tensor_sub(out=d[:, :], in0=negm[:, lo:hi], in1=acc[:, 0:n])
    nc.scalar.activation(out=t[:, :], in_=d[:, :],
                         func=mybir.ActivationFunctionType.Exp)
    nc.vector.tensor_mul(out=t[:, :], in0=t[:, :], in1=acc[:, n:2 * n])
    nc.vector.tensor_add(out=zg[:, :], in0=t[:, :], in1=zloc[:, lo:hi])
    nc.vector.reciprocal(out=cs[:, lo:hi], in_=zg[:, :])


def _emit_body(nc, tc, pools, qT, kT, enc, res, sel, out, use_collective):
    P, ST, PS, OST, DR = pools

    # ---- persistent SBUF tensors -------------------------------
    # qt: one merged [128, 8, 2048] (chunk-major views); v: one merged
    # [128, 16, 1024]; e/kt/part stay per-tile.
    qt_sb = P.tile([128, NDC, S], FP16, tag="qt", name="qt")
    kt_sb = [P.tile([128, S2L], FP16, tag=f"kt{c}", name=f"kt{c}")
             for c in range(NDC)]
    v_sb = P.tile([128, NQT, D], FP16, tag="v", name="v")
    e_sb = [P.tile([128, S2L], FP16, tag=f"e{i}", name=f"e{i}")
            for i in range(NQT)]
    part = [P.tile([128, D], F32, tag=f"pp{i}", name=f"pp{i}")
            for i in range(NKT)]
    negm = P.tile([128, NQT], F32, tag="negm", name="negm")
    zloc = P.tile([128, NQT], F32, tag="zloc", name="zloc")
    cs = P.tile([128, NQT], F32, tag="cs", name="cs")
    sel2_sb = P.tile([128, 2], F32, tag="sel2", name="sel2_sb")
    dummy = P.tile([128, 512], FP16, tag="dummy", name="dummy")

    # ---- PE warm-up: hold the p-state from ~t=0.3us ------------
    # memset runs on Pool immediately; the dummy matmuls then keep the PE
    # busy until the first real operands land (~3.4us), so real matmuls
    # start at the full 2.4 GHz instead of spending their first 3us at
    # 1.2 GHz.
    if NWARM:
        nc.vector.memset(dummy[:, :], 0)
        warm_ps = PS.tile([128, S2L], F32, tag="ps", name="warm")
        for w in range(NWARM):
            nc.tensor.matmul(
                warm_ps[:, 0:256],
                lhsT=dummy[:, 0:128],
                rhs=dummy[:, 0:256],
                start=True, stop=True)

    # ---- load GEMM1 operands (d on partitions, pre-transposed) --
    # kt0-2 are halved across the scalar-HWDGE and Pool-SWDGE queues (Pool
    # DGEs are slow (~1us) but run parallel to the shared HWDGE device) so
    # the ramp staircase's kt chunks land just ahead of their consumption
    # and the first matmul's operands land ~4us; kt3 whole via pool, kt4-7
    # via scalar. qt rides the sync queue, wave 1 strictly before wave 2.
    # The exact queue/halving assignment below was tuned empirically
    # against TimelineSim's serialized DMA device -- it is a carefully
    # balanced arrival schedule; perturbations regress it.
    nc.scalar.dma_start(out=kt_sb[0][:, 0:512], in_=kT[0:128, 0:512])
    nc.gpsimd.dma_start(out=kt_sb[0][:, 512:S2L], in_=kT[0:128, 512:S2L])
    nc.scalar.dma_start(out=kt_sb[1][:, 0:512], in_=kT[128:256, 0:512])
    nc.gpsimd.dma_start(out=kt_sb[1][:, 512:S2L], in_=kT[128:256, 512:S2L])
    nc.gpsimd.dma_start(out=kt_sb[2][:, 0:512], in_=kT[256:384, 0:512])
    nc.scalar.dma_start(out=kt_sb[2][:, 512:S2L], in_=kT[256:384, 512:S2L])
    nc.gpsimd.dma_start(out=kt_sb[3][:, :], in_=kT[384:512, :])
    for c in range(4, NDC):
        nc.scalar.dma_start(out=kt_sb[c][:, :],
                            in_=kT[c * 128:(c + 1) * 128, :])
    # qt wave 1 (first 512 cols of every chunk, feeds the ramp staircase)
    # strictly before wave 2 (cols 512:2048, first needed by the tile-major
    # phase at ~19us) so the serialized DMA device never starves the ramp.
    for c in range(NDC):
        nc.sync.dma_start(out=qt_sb[:, c, 0:512],
                          in_=qT[c * 128:(c + 1) * 128, 0:512])
    for c in range(NDC):
        nc.sync.dma_start(out=qt_sb[:, c, 512:S],
                          in_=qT[c * 128:(c + 1) * 128, 512:S])
    nc.sync.dma_start(out=sel2_sb[:, :], in_=sel)

    # enc/res grouped loads are emitted inside the GEMM1 loop (deadline is
    # the v-scales at ~50us); staging tiles allocated here. pace_t is the
    # target of tiny dep-carrying copies that hold each group's DMA until
    # the qt/kt streams have cleared the serialized DMA device.
    res_st = [ST.tile([128, 4, D], FP16, tag=f"res{g}", name=f"res{g}")
              for g in range(4)]
    pace_t = P.tile([128, 8], FP16, tag="pace", name="pace")

    # ---- GEMM1 + local softmax stats per q tile ----------------
    RAMP = 4  # first tiles run chunk-major so each arriving chunk feeds MMs
    ramp_ps = [PS.tile([128, S2L], F32, tag="ps", name=f"s{qi}")
               for qi in range(RAMP)]
    for s in range(NDC + RAMP - 1):
        # in the arrival-bound first steps, qi descending (chunk ascending)
        # keeps a late-arriving chunk from blocking ready pairs behind it in
        # the in-order PE stream; later steps keep the natural order so
        # tile 0 completes as early as possible
        order = reversed(range(RAMP)) if s < 4 else range(RAMP)
        for qi in order:
            dc = s - qi
            if not 0 <= dc < NDC:
                continue
            for kb in range(NKB):
                nc.tensor.matmul(
                    ramp_ps[qi][:, kb * 512:(kb + 1) * 512],
                    lhsT=qt_sb[:, dc, qi * 128:(qi + 1) * 128],
                    rhs=kt_sb[dc][:, kb * 512:(kb + 1) * 512],
                    start=(dc == 0),
                    stop=(dc == NDC - 1),
                )
    stats_a = None
    acc_a = None
    for qi in range(NQT):
        if qi < RAMP:
            ps = ramp_ps[qi]
        else:
            ps = PS.tile([128, S2L], F32, tag="ps", name=f"s{qi}")
            for dc in range(NDC):
                for kb in range(NKB):
                    nc.tensor.matmul(
                        ps[:, kb * 512:(kb + 1) * 512],
                        lhsT=qt_sb[:, dc, qi * 128:(qi + 1) * 128],
                        rhs=kt_sb[dc][:, kb * 512:(kb + 1) * 512],
                        start=(dc == 0),
                        stop=(dc == NDC - 1),
                    )
        nc.vector.tensor_reduce(
            out=negm[:, qi:qi + 1], in_=ps[:, :],
            axis=mybir.AxisListType.X, op=mybir.AluOpType.max, negate=True)
        # E = exp(S - m_loc) (fp16), Z_loc = row-sum(E) (f32)
        nc.scalar.activation(
            out=e_sb[qi][:, :], in_=ps[:, :],
            func=mybir.ActivationFunctionType.Exp,
            bias=negm[:, qi:qi + 1], scale=1.0,
            accum_out=zloc[:, qi:qi + 1])

        if qi in (1, 3, 8, 10):
            # grouped enc/res loads for tiles [4g, 4g+4), paced behind
            # exp(qi) so their big transfers don't starve the qt/kt streams
            # on the serialized DMA device: enc straight into v_sb via Pool
            # SWDGE, res staged via scalar. DMA issue-hoisting pulls
            # wait-free DMAs to the queue front, so the pace marker is
            # written INTO each DMA's destination: the WAW dependency makes
            # the DMA itself wait on exp(qi). Groups 2/3 are paced late
            # (exp9/exp11) so the DMA device is free for exchange A's sends
            # at ~35us.
            g = {1: 0, 3: 1, 8: 2, 10: 3}[qi]
            nc.vector.tensor_copy(out=v_sb[:, 4 * g, 0:8],
                                  in_=e_sb[qi][:, 0:8])
            nc.gpsimd.dma_start(
                out=v_sb[:, 4 * g:4 * g + 4, :],
                in_=enc.rearrange("(g p) d -> p g d", p=128)[
                    :, 4 * g:4 * g + 4, :])
            nc.scalar.mul(out=res_st[g][:, 0, 0:8], in_=e_sb[qi][:, 8:16],
                          mul=1.0)
            nc.scalar.dma_start(
                out=res_st[g][:, :, :],
                in_=res.rearrange("(g p) d -> p g d", p=128)[
                    :, 4 * g:4 * g + 4, :])

        if qi >= 4:
            # V = enc + res for tile qi-4, on the otherwise-idle Pool engine
            # (keeping the DVE clear for the reduce->exp->exchange chains)
            nc.gpsimd.tensor_add(out=v_sb[:, qi - 4, :],
                                 in0=v_sb[:, qi - 4, :],
                                 in1=res_st[(qi - 4) // 4][:, (qi - 4) % 4, :])

        if qi + 1 == SPLIT:
            # exchange A launches as soon as tile SPLIT-1's stats exist; the
            # collective runs under the rest of GEMM1
            stats_a = _emit_stats_send(nc, P, DR, sel2_sb, negm, zloc, 0,
                                       SPLIT, "a", use_collective)
        if qi == 10:
            # pull exchange A's result in (sync queue reaches this park
            # position well before the collective completes ~54us)
            acc_a = _emit_stats_recv_dma(nc, P, stats_a, SPLIT, "a")
        if qi == 11:
            # merge A + ALL A-half v-scales, queued on DVE after reduce11:
            # the merge parks until cs_a arrives (~57.5) while reduce12-15
            # bypass it through the wait queue, then the scales run just
            # ahead of GEMM2 phase 0's consumption.
            _emit_stats_merge(nc, P, acc_a, negm, zloc, cs, 0, SPLIT, "a")
            for qj in range(SPLIT):
                nc.vector.tensor_scalar_mul(
                    out=v_sb[:, qj, 0:512], in0=v_sb[:, qj, 0:512],
                    scalar1=cs[:, qj:qj + 1])
            for qj in range(4):
                nc.vector.tensor_scalar_mul(
                    out=v_sb[:, qj, 512:D], in0=v_sb[:, qj, 512:D],
                    scalar1=cs[:, qj:qj + 1])
        if qi == 12:
            # second halves for qj 4..7 ride the scalar engine, emitted
            # after exp12 so they sit between exp12 and exp13 in the Act
            # queue (Act is otherwise idle there); deadline is GEMM2
            # ki0.db1 at ~62us.
            for qj in range(4, SPLIT):
                nc.scalar.mul(out=v_sb[:, qj, 512:D],
                              in_=v_sb[:, qj, 512:D],
                              mul=cs[:, qj:qj + 1])

    # final exchange Z: reduce15/exp15 ran at natural times; the mask +
    # send + collective chain starts ~62us and completes under phase 0.
    stats_z = _emit_stats_send(nc, P, DR, sel2_sb, negm, zloc, SPLIT, NQT,
                               "z", use_collective)
    acc_z = _emit_stats_recv_dma(nc, P, stats_z, NQT - SPLIT, "z")
    # trailing V-adds for tiles 12..15 (Pool), emitted after the Z masks so
    # they don't delay the exchange; deadline is the Z v-scales (~80us)
    for qj in range(NQT - 4, NQT):
        nc.gpsimd.tensor_add(out=v_sb[:, qj, :], in0=v_sb[:, qj, :],
                             in1=res_st[qj // 4][:, qj % 4, :])

    # ---- GEMM2: out[k, d] = sum_q E[q, k] * V'[q, d] ------------
    # phase 0: q tiles [0, SPLIT) for ALL ki (only needs exchange A), each
    # [128, D] partial spilled to SBUF; phase 1: q tiles [SPLIT, NQT) with
    # fresh PSUM groups, DVE adds the spill back on the way out.
    for ki in range(NKT):
        psg = PS.tile([128, S2L], F32, tag="ps", name=f"o{ki}")
        for db in range(2):
            for qi in range(SPLIT):
                nc.tensor.matmul(
                    psg[:, db * 512:(db + 1) * 512],
                    lhsT=e_sb[qi][:, ki * 128:(ki + 1) * 128],
                    rhs=v_sb[:, qi, db * 512:(db + 1) * 512],
                    start=(qi == 0),
                    stop=(qi == SPLIT - 1),
                )
        nc.vector.tensor_copy(out=part[ki][:, :], in_=psg[:, :])
        if ki == 3:
            # merge Z + Z-half v-scales; DVE spills for ki4..7 queue after
            # these but their deadlines (phase-1 reads) are far out.
            _emit_stats_merge(nc, P, acc_z, negm, zloc, cs, SPLIT, NQT, "z")
            for qj in range(SPLIT, NQT):
                nc.vector.tensor_scalar_mul(
                    out=v_sb[:, qj, 0:512], in0=v_sb[:, qj, 0:512],
                    scalar1=cs[:, qj:qj + 1])
            for qj in range(SPLIT, SPLIT + 4):
                nc.vector.tensor_scalar_mul(
                    out=v_sb[:, qj, 512:D], in0=v_sb[:, qj, 512:D],
                    scalar1=cs[:, qj:qj + 1])
            for qj in range(SPLIT + 4, NQT):
                nc.scalar.mul(out=v_sb[:, qj, 512:D],
                              in_=v_sb[:, qj, 512:D],
                              mul=cs[:, qj:qj + 1])
    for ki in range(NKT):
        final = ki == NKT - 1
        ot = OST.tile([128, D], FP16, tag="ot", name=f"ot{ki}")
        oq = nc.sync if ki % 2 == 0 else nc.scalar
        if final:
            # last ki accumulates into four independent 256-column PSUM
            # quarters so each quarter's add + store pipelines under the
            # next quarter's matmuls; the adds alternate DVE/Act so the
            # last quarter's add never queues behind the earlier ones
            widths = (320, 320, 256, 128)  # shrinking tail chain
            offs = (0, 320, 640, 896)
            psq = [PS.tile([128, widths[j]], F32, tag="ps", name=f"oq{j}")
                   for j in range(4)]
            for j in range(4):
                lo, w = offs[j], widths[j]
                for qi in range(SPLIT, NQT):
                    nc.tensor.matmul(
                        psq[j][:, :],
                        lhsT=e_sb[qi][:, ki * 128:(ki + 1) * 128],
                        rhs=v_sb[:, qi, lo:lo + w],
                        start=(qi == SPLIT),
                        stop=(qi == NQT - 1),
                    )
                nc.vector.tensor_tensor(
                    out=ot[:, lo:lo + w],
                    in0=psq[j][:, :],
                    in1=part[ki][:, lo:lo + w],
                    op=mybir.AluOpType.add)
                # alternate queues so the quarters' DGEs don't serialize
                (nc.sync if j % 2 == 1 else nc.scalar).dma_start(
                    out=out[ki * 128:(ki + 1) * 128, lo:lo + w],
                    in_=ot[:, lo:lo + w])
            continue
        psg = PS.tile([128, S2L], F32, tag="ps", name=f"o{ki}b")
        for db in range(2):
            for qi in range(SPLIT, NQT):
                nc.tensor.matmul(
                    psg[:, db * 512:(db + 1) * 512],
                    lhsT=e_sb[qi][:, ki * 128:(ki + 1) * 128],
                    rhs=v_sb[:, qi, db * 512:(db + 1) * 512],
                    start=(qi == SPLIT),
                    stop=(qi == NQT - 1),
                )
        nc.vector.tensor_tensor(out=ot[:, :], in0=psg[:, :],
                                in1=part[ki][:, :],
                                op=mybir.AluOpType.add)
        oq.dma_start(out=out[ki * 128:(ki + 1) * 128, :], in_=ot[:, :])


def _build_kernel(nc, qT, kT, enc, res, sel, out, reps=1,
                  use_collective=True):
    tc = tile.TileContext(nc)
    with tc:
        with (
            tc.tile_pool(name="persist", bufs=1) as P,
            tc.tile_pool(name="stage", bufs=1) as ST,
            tc.tile_pool(name="psum", bufs=4, space="PSUM") as PS,
            tc.tile_pool(name="outst", bufs=4) as OST,
            tc.tile_pool(name="dram", bufs=1, space="DRAM") as DR,
        ):
            pools = (P, ST, PS, OST, DR)
            for _ in range(reps):
                _emit_body(nc, tc, pools, qT, kT, enc, res, sel, out,
                           use_collective)
    return nc


def build(reps=1, use_collective=True):
    nc = bacc.Bacc("TRN2", target_bir_lowering=False, debug=False,
                   num_devices=N_CORES)
    qT = nc.dram_tensor("qT", [D, S], FP16, kind="ExternalInput").ap()
    kT = nc.dram_tensor("kT", [D, S2L], FP16, kind="ExternalInput").ap()
    enc = nc.dram_tensor("enc", [S, D], FP16, kind="ExternalInput").ap()
    res = nc.dram_tensor("res", [S, D], FP16, kind="ExternalInput").ap()
    sel = nc.dram_tensor("sel", [128, 2], F32, kind="ExternalInput").ap()
    out = nc.dram_tensor("out", [S2L, D], FP16, kind="ExternalOutput").ap()
    _build_kernel(nc, qT, kT, enc, res, sel, out, reps=reps,
                  use_collective=use_collective)
    nc.compile()
    return nc


def make_in_maps(enc_outputs, atten_outputs, enc_residual):
    enc_outputs = np.asarray(enc_outputs, dtype=np.float32)
    atten_outputs = np.asarray(atten_outputs, dtype=np.float32)
    enc_residual = np.asarray(enc_residual, dtype=np.float32)
    enc16 = enc_outputs.astype(np.float16)
    att16 = atten_outputs.astype(np.float16)
    res16 = enc_residual.astype(np.float16)
    in_maps = []
    for core in range(N_CORES):
        b, half = core // 2, core % 2
        sel = np.zeros((128, 2), np.float32)
        sel[:, (core & 1) ^ 1] = 1.0
        in_maps.append({
            "qT": np.ascontiguousarray(enc16[b].T),
            "kT": np.ascontiguousarray(att16[b, half * S2L:(half + 1) * S2L, :].T),
            "enc": enc16[b],
            "res": res16[b],
            "sel": sel,
        })
    return in_maps


def assemble(results):
    out = np.empty((B, S, D), np.float32)
    for core in range(N_CORES):
        b, half = core // 2, core % 2
        out[b, half * S2L:(half + 1) * S2L, :] = \
            results[core]["out"].astype(np.float32)
    return out


_NC = None


def kernel(enc_outputs, atten_outputs, enc_residual):
    global _NC
    if _NC is None:
        _NC = build()
    in_maps = make_in_maps(enc_outputs, atten_outputs, enc_residual)
    last_err = None
    for _attempt in range(3):
        try:
            res = run_bass_kernel_spmd(_NC, in_maps,
                                       core_ids=list(range(N_CORES)))
            return assemble(res.results)
        except Exception as e:  # transient device/tunnel errors -- retry
            last_err = e
    raise last_err
